# revision 3
# baseline (speedup 1.0000x reference)
"""MinGRU Trainium2 kernel — mixed-precision matmul (bf16 + fp8 DoubleRow).

Full-input contract: kernel(x=[8,4096,1024] f32, W_hg=[2048,1024] f32)
-> [8,4096,1024] f32. Data-parallel over batch: core i owns example i.

Measured hw facts driving this design (this container, axon trn2):
  - PE fp32r / bf16: 1 moving column per cycle per 128-row k-tile.
  - PE fp8 DoubleRow: 1 column pair per cycle per 256-row k-pair (2x,
    157 TF/s; the 4x in the CoreSim cost model is NOT real — verified
    with matmul-only variants at SC=512 and SC=256).
  - Pointwise+scan pipeline alone: ~52 us — far under the PE floor.

Precision scheme (CPU-sim rel_fro, hw matches sim to 4+ digits):
  - gate path: 1-pass fp8 e4m3 (sigmoid damps quantization error)
  - hidden path: first M8 k-tiles fp8 DoubleRow, rest bf16
      M8=0: 164 us PE floor, 8.1e-3   M8=4: 136.5 us, 1.57e-2
  - all operands pre-scaled by 32 on host (bf16 too, so fp8 and bf16
    products accumulate in PSUM at a common 1024x scale); the 1/1024
    is folded into the ScalarE activation scales.

Pointwise per [128,512] tile (PSUM holds 1024x-scaled values):
    a    = sigmoid(-pg/1024)      ScalarE
    sh   = sigmoid(ph/1024)       ScalarE
    r    = relu(ph/1024)          ScalarE
    gt   = min(sh,0.5) + r        DVE stt
    bneg = (a-1)*gt               DVE stt
    h_t  = a*h_{t-1} - bneg_t     DVE tensor_tensor_scan (fp32 state)

Loop order: seq-chunk OUTER, weights resident, x streamed per chunk,
triple-buffered x and PSUM so DMA (incl. the next For_i iteration's
loads) overlaps compute (the db-outer resident-x layout serializes
~40 us of x reload at iteration boundaries).
"""

from contextlib import ExitStack

import numpy as np

B, S, D = 8, 4096, 1024
P = 128
KK = D // P  # 8 contraction sub-tiles of 128
KP = KK // 2  # 4 DoubleRow k-pairs
DB = D // P  # 8 output-channel blocks
SC = 512
NSC = S // SC
M8 = 4  # hidden k-tiles (of 8) computed in fp8 DoubleRow; rest bf16
SCALE = 32.0  # per-operand scale; 1/1024 folded into activation scales
ISCL = 1.0 / (SCALE * SCALE)

_NC_CACHE = {}


def _build_bass(
    repeat=1, loop_repeat=None, psum_bufs=3, x_bufs=3, m8=M8, mode="full"
):
    import contextlib

    import concourse.tile as tile
    from concourse import bacc, mybir

    f32 = mybir.dt.float32
    bf16 = mybir.dt.bfloat16
    f8 = mybir.dt.float8e4
    AF = mybir.ActivationFunctionType
    OP = mybir.AluOpType
    DR = mybir.MatmulPerfMode.DoubleRow

    assert m8 % 2 == 0 and 0 <= m8 <= KK
    nb = KK - m8  # bf16 hidden k-tiles

    nc = bacc.Bacc("TRN2", debug=False)
    xb = nc.dram_tensor("xb", [D, S], bf16, kind="ExternalInput").ap()
    xhi = nc.dram_tensor("xhi", [D, S], f8, kind="ExternalInput").ap()
    whb = nc.dram_tensor("whb", [D, D], bf16, kind="ExternalInput").ap()
    wh8 = nc.dram_tensor("wh8", [D, D], f8, kind="ExternalInput").ap()
    wg8 = nc.dram_tensor("wg8", [D, D], f8, kind="ExternalInput").ap()
    out = nc.dram_tensor("out", [D, S], f32, kind="ExternalOutput").ap()

    xb_v = xb.rearrange("(kk p) s -> p kk s", p=P)
    xhi_v = xhi.rearrange("(kk p) s -> p kk s", p=P)
    whb_v = whb.rearrange("(kk p) e -> p kk e", p=P)
    wh8_v = wh8.rearrange("(kk p) e -> p kk e", p=P)
    wg8_v = wg8.rearrange("(kk p) e -> p kk e", p=P)

    with tile.TileContext(nc) as tc, ExitStack() as ctx:
        xpool = ctx.enter_context(tc.tile_pool(name="x", bufs=x_bufs))
        wpool = ctx.enter_context(tc.tile_pool(name="w", bufs=2))
        ppool = ctx.enter_context(
            tc.tile_pool(name="ps", bufs=psum_bufs, space="PSUM")
        )
        spool = ctx.enter_context(tc.tile_pool(name="s", bufs=3))
        opool = ctx.enter_context(tc.tile_pool(name="o", bufs=2))

        loop_cm = (
            tc.For_i(0, loop_repeat, 1)
            if loop_repeat is not None
            else contextlib.nullcontext()
        )
        with loop_cm:
            for _rep in range(repeat):
                # weights resident for the whole pass
                wh = None
                w8 = None
                if nb:
                    wh = wpool.tile([P, nb, D], bf16, tag="wh")
                    nc.sync.dma_start(wh[:], whb_v[:, m8:, :])
                if m8:
                    w8 = wpool.tile([P, m8, D], f8, tag="w8")
                    nc.sync.dma_start(w8[:], wh8_v[:, :m8, :])
                wg = wpool.tile([P, KK, D], f8, tag="wg")
                nc.sync.dma_start(wg[:], wg8_v[:, :, :])

                prev_o = [None] * DB
                for sc in range(NSC):
                    s0, s1 = sc * SC, (sc + 1) * SC
                    xbt = None
                    if nb:
                        xbt = xpool.tile([P, nb, SC], bf16, tag="xb")
                        nc.sync.dma_start(xbt[:], xb_v[:, m8:, s0:s1])
                    x8t = xpool.tile([P, KK, SC], f8, tag="x8")
                    nc.sync.dma_start(x8t[:], xhi_v[:, :, s0:s1])

                    for db in range(DB):
                        e0 = db * P
                        pg = ppool.tile([P, SC], f32, tag="pg")
                        for kp in range(KP):
                            nc.tensor.matmul(
                                pg[:],
                                wg[:, 2 * kp : 2 * kp + 2, e0 : e0 + P],
                                x8t[:, 2 * kp : 2 * kp + 2, :],
                                start=(kp == 0),
                                stop=(kp == KP - 1),
                                perf_mode=DR,
                            )
                        ph = ppool.tile([P, SC], f32, tag="ph")
                        nmm = m8 // 2 + nb
                        mm = 0
                        for kp in range(m8 // 2):
                            nc.tensor.matmul(
                                ph[:],
                                w8[:, 2 * kp : 2 * kp + 2, e0 : e0 + P],
                                x8t[:, 2 * kp : 2 * kp + 2, :],
                                start=(mm == 0),
                                stop=(mm == nmm - 1),
                                perf_mode=DR,
                            )
                            mm += 1
                        for k in range(nb):
                            nc.tensor.matmul(
                                ph[:],
                                wh[:, k, e0 : e0 + P],
                                xbt[:, k, :],
                                start=(mm == 0),
                                stop=(mm == nmm - 1),
                            )
                            mm += 1

                        if mode == "mm":
                            oc = opool.tile([P, SC], f32, tag=f"oc{db}")
                            nc.scalar.activation(oc[:], ph[:], AF.Copy)
                            oc2 = opool.tile([P, SC], f32, tag=f"od{db}")
                            nc.scalar.activation(oc2[:], pg[:], AF.Copy)
                            nc.sync.dma_start(out[e0 : e0 + P, s0:s1], oc[:])
                            continue

                        a = spool.tile([P, SC], f32, tag="a")
                        nc.scalar.activation(a[:], pg[:], AF.Sigmoid, scale=-ISCL)
                        sh = spool.tile([P, SC], f32, tag="sh")
                        nc.scalar.activation(sh[:], ph[:], AF.Sigmoid, scale=ISCL)
                        r = spool.tile([P, SC], f32, tag="r")
                        nc.scalar.activation(r[:], ph[:], AF.Relu, scale=ISCL)
                        gt = spool.tile([P, SC], f32, tag="gt")
                        nc.vector.scalar_tensor_tensor(
                            gt[:], sh[:], 0.5, r[:], op0=OP.min, op1=OP.add
                        )
                        bn = spool.tile([P, SC], f32, tag="bn")
                        nc.vector.scalar_tensor_tensor(
                            bn[:], a[:], 1.0, gt[:], op0=OP.subtract, op1=OP.mult
                        )

                        o = opool.tile([P, SC], f32, tag=f"o{db}")
                        init = 0.0 if sc == 0 else prev_o[db][:, SC - 1 : SC]
                        nc.vector.tensor_tensor_scan(
                            o[:], a[:], bn[:], init, op0=OP.mult, op1=OP.subtract
                        )
                        prev_o[db] = o
                        nc.sync.dma_start(out[e0 : e0 + P, s0:s1], o[:])
    nc.compile()
    return nc


def _get_nc():
    if "nc" not in _NC_CACHE:
        _NC_CACHE["nc"] = _build_bass()
    return _NC_CACHE["nc"]


def _run(in_maps, trace=False, **kw):
    from concourse import bass_utils

    nc = _get_nc()
    return bass_utils.run_bass_kernel_spmd(
        nc, in_maps, core_ids=list(range(B)), trace=trace, **kw
    )


def _make_in_maps(x, W_hg):
    import ml_dtypes

    E4 = ml_dtypes.float8_e4m3
    BF = ml_dtypes.bfloat16
    x = np.asarray(x, dtype=np.float32)
    W = np.asarray(W_hg, dtype=np.float32)

    whT = np.ascontiguousarray(SCALE * W[:D].T)  # [D, D] f32, scaled
    wgT = np.ascontiguousarray(SCALE * W[D:].T)
    whb = whT.astype(BF)
    wh8 = whT.astype(E4)
    wg8 = wgT.astype(E4)

    maps = []
    for i in range(B):
        xs = np.ascontiguousarray(SCALE * x[i].T)  # [D, S] f32, scaled
        maps.append(
            {
                "xb": xs.astype(BF),
                "xhi": xs.astype(E4),
                "whb": whb,
                "wh8": wh8,
                "wg8": wg8,
            }
        )
    return maps


def kernel(x, W_hg):
    res = _run(_make_in_maps(x, W_hg))
    outs = [r["out"] for r in res.results]
    return np.stack([o.T for o in outs], axis=0).astype(np.float32)


# revision 4
# speedup vs baseline: 1.5129x; 1.5129x over previous
"""MinGRU Trainium2 kernel — mixed-precision matmul (bf16 + fp8 DoubleRow).

Full-input contract: kernel(x=[8,4096,1024] f32, W_hg=[2048,1024] f32)
-> [8,4096,1024] f32. Data-parallel over batch: core i owns example i.

Measured hw facts driving this design (this container, axon trn2):
  - PE fp32r / bf16: 1 moving column per cycle per 128-row k-tile.
  - PE fp8 DoubleRow: 1 column pair per cycle per 256-row k-pair (2x,
    157 TF/s; the 4x in the CoreSim cost model is NOT real — verified
    with matmul-only variants at SC=512 and SC=256).
  - Pointwise+scan pipeline alone: ~52 us — far under the PE floor.

Precision scheme (CPU-sim rel_fro, hw matches sim to 4+ digits):
  - gate path: 1-pass fp8 e4m3 (sigmoid damps quantization error)
  - hidden path: first M8 k-tiles fp8 DoubleRow, rest bf16
      M8=0: 164 us PE floor, 8.1e-3   M8=4: 136.5 us, 1.57e-2
  - all operands pre-scaled by 32 on host (bf16 too, so fp8 and bf16
    products accumulate in PSUM at a common 1024x scale); the 1/1024
    is folded into the ScalarE activation scales.

Pointwise per [128,512] tile (PSUM holds 1024x-scaled values):
    a    = sigmoid(-pg/1024)      ScalarE
    sh   = sigmoid(ph/1024)       ScalarE
    r    = relu(ph/1024)          ScalarE
    gt   = min(sh,0.5) + r        DVE stt
    bneg = (a-1)*gt               DVE stt
    h_t  = a*h_{t-1} - bneg_t     DVE tensor_tensor_scan (fp32 state)

Loop order: seq-chunk OUTER, weights resident, x streamed per chunk,
triple-buffered x and PSUM so DMA (incl. the next For_i iteration's
loads) overlaps compute (the db-outer resident-x layout serializes
~40 us of x reload at iteration boundaries).
"""

from contextlib import ExitStack

import numpy as np

B, S, D = 8, 4096, 1024
P = 128
KK = D // P  # 8 contraction sub-tiles of 128
KP = KK // 2  # 4 DoubleRow k-pairs
DB = D // P  # 8 output-channel blocks
SC = 512
NSC = S // SC
# Hidden k-tiles (of 8) computed in fp8 DoubleRow; rest bf16. M8=4 has a
# 136.5 us PE floor (rel 1.57e-2) but measures 201 us: PE group-transition
# stalls eat the gain whenever per-unit PE time drops below the consumer
# chain latency. M8=0 (pure bf16 hidden) measures AT its floor: 161 us,
# rel 8.1e-3 — strictly better on both axes, so it is the default.
M8 = 0
SCALE = 32.0  # per-operand scale; 1/1024 folded into activation scales
ISCL = 1.0 / (SCALE * SCALE)

_NC_CACHE = {}


def _build_bass(
    repeat=1, loop_repeat=None, psum_bufs=3, x_bufs=3, m8=M8, mode="full"
):
    import contextlib

    import concourse.tile as tile
    from concourse import bacc, mybir

    f32 = mybir.dt.float32
    bf16 = mybir.dt.bfloat16
    f8 = mybir.dt.float8e4
    AF = mybir.ActivationFunctionType
    OP = mybir.AluOpType
    DR = mybir.MatmulPerfMode.DoubleRow

    assert m8 % 2 == 0 and 0 <= m8 <= KK
    nb = KK - m8  # bf16 hidden k-tiles

    nc = bacc.Bacc("TRN2", debug=False)
    xb = nc.dram_tensor("xb", [D, S], bf16, kind="ExternalInput").ap()
    xhi = nc.dram_tensor("xhi", [D, S], f8, kind="ExternalInput").ap()
    whb = nc.dram_tensor("whb", [D, D], bf16, kind="ExternalInput").ap()
    wh8 = nc.dram_tensor("wh8", [D, D], f8, kind="ExternalInput").ap()
    wg8 = nc.dram_tensor("wg8", [D, D], f8, kind="ExternalInput").ap()
    out = nc.dram_tensor("out", [D, S], f32, kind="ExternalOutput").ap()

    xb_v = xb.rearrange("(kk p) s -> p kk s", p=P)
    xhi_v = xhi.rearrange("(kk p) s -> p kk s", p=P)
    whb_v = whb.rearrange("(kk p) e -> p kk e", p=P)
    wh8_v = wh8.rearrange("(kk p) e -> p kk e", p=P)
    wg8_v = wg8.rearrange("(kk p) e -> p kk e", p=P)

    with tile.TileContext(nc) as tc, ExitStack() as ctx:
        xpool = ctx.enter_context(tc.tile_pool(name="x", bufs=x_bufs))
        wpool = ctx.enter_context(tc.tile_pool(name="w", bufs=2))
        ppool = ctx.enter_context(
            tc.tile_pool(name="ps", bufs=psum_bufs, space="PSUM")
        )
        spool = ctx.enter_context(tc.tile_pool(name="s", bufs=3))
        opool = ctx.enter_context(tc.tile_pool(name="o", bufs=2))

        loop_cm = (
            tc.For_i(0, loop_repeat, 1)
            if loop_repeat is not None
            else contextlib.nullcontext()
        )
        with loop_cm:
            for _rep in range(repeat):
                # weights resident for the whole pass
                wh = None
                w8 = None
                if nb:
                    wh = wpool.tile([P, nb, D], bf16, tag="wh")
                    nc.sync.dma_start(wh[:], whb_v[:, m8:, :])
                if m8:
                    w8 = wpool.tile([P, m8, D], f8, tag="w8")
                    nc.sync.dma_start(w8[:], wh8_v[:, :m8, :])
                wg = wpool.tile([P, KK, D], f8, tag="wg")
                nc.sync.dma_start(wg[:], wg8_v[:, :, :])

                prev_o = [None] * DB
                for sc in range(NSC):
                    s0, s1 = sc * SC, (sc + 1) * SC
                    xbt = None
                    if nb:
                        xbt = xpool.tile([P, nb, SC], bf16, tag="xb")
                        nc.sync.dma_start(xbt[:], xb_v[:, m8:, s0:s1])
                    x8t = xpool.tile([P, KK, SC], f8, tag="x8")
                    nc.sync.dma_start(x8t[:], xhi_v[:, :, s0:s1])

                    for db in range(DB):
                        e0 = db * P
                        pg = ppool.tile([P, SC], f32, tag="pg")
                        for kp in range(KP):
                            nc.tensor.matmul(
                                pg[:],
                                wg[:, 2 * kp : 2 * kp + 2, e0 : e0 + P],
                                x8t[:, 2 * kp : 2 * kp + 2, :],
                                start=(kp == 0),
                                stop=(kp == KP - 1),
                                perf_mode=DR,
                            )
                        ph = ppool.tile([P, SC], f32, tag="ph")
                        nmm = m8 // 2 + nb
                        mm = 0
                        for kp in range(m8 // 2):
                            nc.tensor.matmul(
                                ph[:],
                                w8[:, 2 * kp : 2 * kp + 2, e0 : e0 + P],
                                x8t[:, 2 * kp : 2 * kp + 2, :],
                                start=(mm == 0),
                                stop=(mm == nmm - 1),
                                perf_mode=DR,
                            )
                            mm += 1
                        for k in range(nb):
                            nc.tensor.matmul(
                                ph[:],
                                wh[:, k, e0 : e0 + P],
                                xbt[:, k, :],
                                start=(mm == 0),
                                stop=(mm == nmm - 1),
                            )
                            mm += 1

                        if mode == "mm":
                            oc = opool.tile([P, SC], f32, tag=f"oc{db}")
                            nc.scalar.activation(oc[:], ph[:], AF.Copy)
                            oc2 = opool.tile([P, SC], f32, tag=f"od{db}")
                            nc.scalar.activation(oc2[:], pg[:], AF.Copy)
                            nc.sync.dma_start(out[e0 : e0 + P, s0:s1], oc[:])
                            continue

                        a = spool.tile([P, SC], f32, tag="a")
                        nc.scalar.activation(a[:], pg[:], AF.Sigmoid, scale=-ISCL)
                        sh = spool.tile([P, SC], f32, tag="sh")
                        nc.scalar.activation(sh[:], ph[:], AF.Sigmoid, scale=ISCL)
                        r = spool.tile([P, SC], f32, tag="r")
                        nc.scalar.activation(r[:], ph[:], AF.Relu, scale=ISCL)
                        gt = spool.tile([P, SC], f32, tag="gt")
                        nc.vector.scalar_tensor_tensor(
                            gt[:], sh[:], 0.5, r[:], op0=OP.min, op1=OP.add
                        )
                        bn = spool.tile([P, SC], f32, tag="bn")
                        nc.vector.scalar_tensor_tensor(
                            bn[:], a[:], 1.0, gt[:], op0=OP.subtract, op1=OP.mult
                        )

                        o = opool.tile([P, SC], f32, tag=f"o{db}")
                        init = 0.0 if sc == 0 else prev_o[db][:, SC - 1 : SC]
                        nc.vector.tensor_tensor_scan(
                            o[:], a[:], bn[:], init, op0=OP.mult, op1=OP.subtract
                        )
                        prev_o[db] = o
                        nc.sync.dma_start(out[e0 : e0 + P, s0:s1], o[:])
    nc.compile()
    return nc


def _get_nc():
    if "nc" not in _NC_CACHE:
        _NC_CACHE["nc"] = _build_bass()
    return _NC_CACHE["nc"]


def _run(in_maps, trace=False, **kw):
    from concourse import bass_utils

    nc = _get_nc()
    return bass_utils.run_bass_kernel_spmd(
        nc, in_maps, core_ids=list(range(B)), trace=trace, **kw
    )


def _make_in_maps(x, W_hg):
    import ml_dtypes

    E4 = ml_dtypes.float8_e4m3
    BF = ml_dtypes.bfloat16
    x = np.asarray(x, dtype=np.float32)
    W = np.asarray(W_hg, dtype=np.float32)

    whT = np.ascontiguousarray(SCALE * W[:D].T)  # [D, D] f32, scaled
    wgT = np.ascontiguousarray(SCALE * W[D:].T)
    whb = whT.astype(BF)
    wh8 = whT.astype(E4)
    wg8 = wgT.astype(E4)

    maps = []
    for i in range(B):
        xs = np.ascontiguousarray(SCALE * x[i].T)  # [D, S] f32, scaled
        maps.append(
            {
                "xb": xs.astype(BF),
                "xhi": xs.astype(E4),
                "whb": whb,
                "wh8": wh8,
                "wg8": wg8,
            }
        )
    return maps


def kernel(x, W_hg):
    res = _run(_make_in_maps(x, W_hg))
    outs = [r["out"] for r in res.results]
    return np.stack([o.T for o in outs], axis=0).astype(np.float32)
